# revision 17
# baseline (speedup 1.0000x reference)
"""Trainium2 Bass kernel for nn_ANN_Comp_29240137351521 (dense_cnn).

Reference computes, per batch row b of x [16384, 512] (complex, given as
real/imag f32 pairs):
    h = x @ w0                      # [B, 512] complex
    a = ifft(fft(h, n=1023)^2)      # full self-convolution, [B, 1023]
    out = |a @ wlast|               # [B, 10] f32

Algebraic collapse used here: the self-convolution + final contraction is a
polynomial-evaluation identity. With L = 1024 >= 2*512-1 evaluation points at
the L-th roots of unity:
    e   = x @ F        where F  = fft(w0, n=L, axis=1)        [512, L]
    z   = (e*e) @ Wt   where Wt = ifft(pad(wlast, L), axis=0) [L, 10]
    out = |z|
so the whole network is two dense matmuls + an elementwise complex square --
no FFT on device. F and Wt are tiny weight transforms folded on the host.

Real-expanded form on device (per core, data-parallel over batch), Gauss
3-multiplication split of the complex matmul; everything transposed (l on
partitions, batch free):
    P1 = xr@Fr ; P2 = xi@Fi ; P3 = (xr+xi)@(Fr+Fi)    (PSUM accumulation)
    m = 2*P1-P3 = er-ei ;  p = P3-2*P2 = er+ei        (DVE fused ops)
    s = p*m  = Re e^2                                 (DVE mult, bf16)
    a = p^2 ; b = m^2  (ACT squares, bf16)
    t = a-b  = 4*er*ei = 2*Im e^2                     (DVE 16-bit subtract)
    z += s@[Wtr|Wti] + t@[-Wti/2|Wtr/2]               (second matmul)
    host: out = sqrt(zr^2 + zi^2)

Scheduling notes (measured-trace rationale):
  * z-stage is 4x column-tiled: the z weights are only 20 real columns, so
    four z matmuls (s/t for an l-pair) run CONCURRENTLY in the four
    32-column groups of the PE array via tile_position=(0, 32j), each
    accumulating into its own partition quadrant of one PSUM bank (pack
    span measured 400ns vs 864ns serial).  Host adds the quadrants.
  * Streams run P3,P1,P2 per tile so the P3 PSUM->SBUF copy (ACT) hides
    under the P1/P2 matmuls and the final tile's DVE chain is ~2us
    shorter.
  * F_2 is never DMA'd: the DVE derives f2 = f3 - f1 chunk by chunk in
    otherwise-idle slots (~0.33us each).  That removes 1MB from the load
    stream, which is what lets the two ~140GB/s HWDGE lanes feed the
    b0 ramp, the F tails AND batch-1's x without missing a deadline.
  * The b0 x streams are quarter-split (128KB d-slices) and interleaved
    across both HW lanes so they land at the aggregate rate; P3 starts
    on d0 alone ~1.5us after the queues open.
  * gpsimd software-DGE completion semaphores lag the transfer ~5us, so
    it only carries late-consumed data (xti -- the P2 stream runs last
    -- plus z-weights) and the dep-gated bulk x for batches 2/3; an
    ungated bulk stream would dilute the HW lanes during the head (the
    DMA engines share bandwidth across active queues).
  * b0 is emitted as a 2-tile software-pipelined prologue
    (P3/P1 of tiles 0,1 before any P2) so the PE isn't head-of-line
    blocked on the laggy xti semaphore.
  * The final batch's last l-pair flushes as two half-packs: the l6 half
    right after tile7's stage-1 matmuls (copy+DMA of PSUM quadrants 0/1
    overlap tile7's DVE chain), the l7 half right after the final DVE
    chain.

Sharding: pure data parallel -- batch split 8 ways, weights replicated.
"""

import numpy as np
import ml_dtypes

import concourse.bass as bass
import concourse.mybir as mybir
from concourse import bacc, tile
from concourse.bass_utils import run_bass_kernel_spmd

NCORES = 8
B, D, L, C = 16384, 512, 1024, 10
BC = B // NCORES
P = 128
BN = 512
ND = D // P
NL = L // P
NB = BC // BN
WZ = 32                  # padded z-weight columns per l-chunk (col-tiling)

F32 = mybir.dt.float32
BF16 = mybir.dt.bfloat16
ALU = mybir.AluOpType

_NC_CACHE = None


def build_nc():
    global _NC_CACHE
    if _NC_CACHE is not None:
        return _NC_CACHE

    nc = bacc.Bacc(None, target_bir_lowering=False)

    xtr_d = nc.declare_dram_parameter("xT_r", [P, NB, ND * BN], BF16,
                                      isOutput=False)
    xti_d = nc.declare_dram_parameter("xT_i", [P, NB, ND * BN], BF16,
                                      isOutput=False)
    xts_d = nc.declare_dram_parameter("xT_s", [P, NB, ND * BN], BF16,
                                      isOutput=False)
    f1_d = nc.declare_dram_parameter("F_1", [P, ND * L], BF16, isOutput=False)
    f3_d = nc.declare_dram_parameter("F_3", [P, ND * L], BF16, isOutput=False)
    # z-weights: [l-part 128, NL*32]: per l-chunk 20 real columns zero-padded
    # to 32 so four of them tile the PE array's four column groups.
    w1_d = nc.declare_dram_parameter("W_1", [P, NL * WZ], BF16, isOutput=False)
    w2_d = nc.declare_dram_parameter("W_2", [P, NL * WZ], BF16, isOutput=False)
    # out: 4 partition quadrants x [zr(10)|zi(10)|pad(12)] x batch; host sums
    # the quadrants.
    out_d = nc.declare_dram_parameter("out", [P, BC], F32, isOutput=True)

    with tile.TileContext(nc) as tc:
        with (
            tc.tile_pool(name="wts", bufs=1) as wts,
            tc.tile_pool(name="xs", bufs=1) as xs,
            tc.tile_pool(name="tmp", bufs=3) as tmp,
            tc.tile_pool(name="sqf", bufs=3) as sqf,
            tc.tile_pool(name="sq", bufs=5) as sq,
            tc.tile_pool(name="zo", bufs=2) as zo,
            tc.tile_pool(name="pse", bufs=2, space="PSUM") as pse,
            tc.tile_pool(name="psz", bufs=2, space="PSUM") as psz,
        ):
            # PE warm-up (releases the HAM clock gate).
            dummy = wts.tile([P, 64], BF16, tag="dummy")
            nc.gpsimd.memset(dummy[:], 0.0)
            wacc = pse.tile([64, 64], F32, tag="p1")
            for i in range(12):
                nc.tensor.matmul(wacc[:], dummy[:, 0:64], dummy[:],
                                 start=(i == 0), stop=False,
                                 skip_group_check=True)

            def warm_fill(n):
                for _ in range(n):
                    nc.tensor.matmul(wacc[:], dummy[:, 0:64], dummy[:],
                                     start=False, stop=False,
                                     skip_group_check=True)

            f1 = wts.tile([P, ND * L], BF16, tag="f1")
            f2 = wts.tile([P, ND * L], BF16, tag="f2")
            f3 = wts.tile([P, ND * L], BF16, tag="f3")
            xtr = xs.tile([P, NB, ND * BN], BF16, tag="xtr")
            xti = xs.tile([P, NB, ND * BN], BF16, tag="xti")
            xts = xs.tile([P, NB, ND * BN], BF16, tag="xts")
            w1 = wts.tile([P, NL * WZ], BF16, tag="w1")
            w2 = wts.tile([P, NL * WZ], BF16, tag="w2")

            def flc(l):         # one l-chunk of F (l-major): 128KB
                return slice(l * D, (l + 1) * D)

            def dsl(d):         # one d-slice of a packed x batch chunk
                return slice(d * BN, (d + 1) * BN)

            # sync lane: xts/xtr even d-slices, f1 chunks, b1's P3/P1
            # inputs woven between the f1 tails.
            nc.sync.dma_start(xts[:, 0, dsl(0)], xts_d[:, 0, dsl(0)])
            nc.sync.dma_start(xts[:, 0, dsl(2)], xts_d[:, 0, dsl(2)])
            nc.sync.dma_start(f1[:, flc(0)], f1_d[:, flc(0)])
            nc.sync.dma_start(xtr[:, 0, dsl(0)], xtr_d[:, 0, dsl(0)])
            nc.sync.dma_start(xtr[:, 0, dsl(2)], xtr_d[:, 0, dsl(2)])
            nc.sync.dma_start(f1[:, flc(1)], f1_d[:, flc(1)])
            for l in range(2, NL):
                nc.sync.dma_start(f1[:, flc(l)], f1_d[:, flc(l)])
            nc.sync.dma_start(xtr[:, 1, 0:2 * BN], xtr_d[:, 1, 0:2 * BN])
            nc.sync.dma_start(xtr[:, 1, 2 * BN:], xtr_d[:, 1, 2 * BN:])
            # scalar lane (= ACT engine, 10 dispatches ~7us -- it must be
            # free for the first PSUM->SBUF copy by ~14us): f3 chunks +
            # xts/xtr odd d-slices.
            nc.scalar.dma_start(f3[:, flc(0)], f3_d[:, flc(0)])
            nc.scalar.dma_start(xts[:, 0, dsl(1)], xts_d[:, 0, dsl(1)])
            nc.scalar.dma_start(xts[:, 0, dsl(3)], xts_d[:, 0, dsl(3)])
            nc.scalar.dma_start(xtr[:, 0, dsl(1)], xtr_d[:, 0, dsl(1)])
            nc.scalar.dma_start(xtr[:, 0, dsl(3)], xtr_d[:, 0, dsl(3)])
            nc.scalar.dma_start(f3[:, flc(1)], f3_d[:, flc(1)])
            nc.scalar.dma_start(f3[:, flc(2)], f3_d[:, flc(2)])
            nc.scalar.dma_start(f3[:, flc(3)], f3_d[:, flc(3)])
            nc.scalar.dma_start(f3[:, flc(4).start:flc(5).stop],
                                f3_d[:, flc(4).start:flc(5).stop])
            nc.scalar.dma_start(f3[:, flc(6).start:flc(7).stop],
                                f3_d[:, flc(6).start:flc(7).stop])
            nc.scalar.dma_start(xts[:, 1, 0:2 * BN], xts_d[:, 1, 0:2 * BN])
            nc.scalar.dma_start(xts[:, 1, 2 * BN:], xts_d[:, 1, 2 * BN:])
            # gpsimd SW-DGE: late-consumed data only (see module docstring)
            nc.gpsimd.dma_start(xti[:, 0, 0:2 * BN], xti_d[:, 0, 0:2 * BN])
            nc.gpsimd.dma_start(xti[:, 0, 2 * BN:], xti_d[:, 0, 2 * BN:])
            nc.gpsimd.dma_start(w1[:], w1_d[:])
            nc.gpsimd.dma_start(w2[:], w2_d[:])

            def late_x(bstreams, dep):
                # Delay the software-DGE dispatch of bulk x until `dep` (a
                # mid-stream compute tile) exists: a dep-gated byte write
                # into each destination slice makes the DMA wait via WAW
                # ordering.
                for xt, xd, b in bstreams:
                    nc.gpsimd.tensor_copy(xt[:, b, 0:1], dep)
                    nc.gpsimd.dma_start(xt[:, b, :], xd[:, b, :])

            def derive_f2(l):   # f2 chunk = f3 - f1 on the DVE (bf16 2x)
                nc.vector.tensor_sub(f2[:, flc(l)], f3[:, flc(l)],
                                     f1[:, flc(l)])

            def fsl(d, l):      # F weight chunk (d, l) in l-major packing
                return slice(l * D + d * P, l * D + (d + 1) * P)

            def wsl(l):         # z-weight slice for l-chunk (32 cols)
                return slice(l * WZ, (l + 1) * WZ)

            # z-stage: per batch, 4 packs; pack i contracts the l-pair
            # (2i, 2i+1): [s_2i@W1, t_2i@W2, s_2i+1@W1, t_2i+1@W2] run
            # concurrently in the 4 column groups of the PE array,
            # accumulating into partition quadrants 0..3 of one PSUM bank.
            # Host adds the quadrants.
            packs = []     # queued (zz, b, i, (s0,t0,s1,t1), bs)

            def zpack(zz, b, i, st, bs):
                s0, t0, s1, t1 = st
                for j, (wt, rhs, l) in enumerate((
                        (w1, s0, 2 * i), (w2, t0, 2 * i),
                        (w1, s1, 2 * i + 1), (w2, t1, 2 * i + 1))):
                    nc.tensor.matmul(
                        zz[32 * j:32 * j + 32, :], wt[:, wsl(l)], rhs[:],
                        start=(i == 0), stop=(i == NL // 2 - 1),
                        tile_position=(0, 32 * j),
                        skip_group_check=True)
                if i == NL // 2 - 1:
                    # copy+DMA in column halves so they overlap
                    zt = zo.tile([P, BN], F32, tag="zt")
                    half = BN // 2
                    nc.scalar.copy(zt[:, 0:half], zz[:, 0:half])
                    nc.sync.dma_start(
                        out_d[:, bs.start:bs.start + half], zt[:, 0:half])
                    nc.scalar.copy(zt[:, half:], zz[:, half:])
                    nc.sync.dma_start(
                        out_d[:, bs.start + half:bs.stop], zt[:, half:])

            def mm_stream(dst, ft, xt, b, l):
                for d in range(ND):
                    nc.tensor.matmul(
                        dst[:], ft[:, fsl(d, l)], xt[:, b, dsl(d)],
                        start=(d == 0), stop=(d == ND - 1),
                        skip_group_check=True)

            def dve_chain(p1, p2, c3):
                """m/p/s/a/bq/t from the PSUM pair + the already-copied c3
                (the copy runs on ACT during the P1/P2 matmuls)."""
                m = tmp.tile([P, BN], F32, tag="m")
                nc.vector.scalar_tensor_tensor(
                    m[:], p1[:], 2.0, c3[:], ALU.mult, ALU.subtract)
                p = tmp.tile([P, BN], F32, tag="p")
                nc.vector.scalar_tensor_tensor(
                    p[:], p2[:], -2.0, c3[:], ALU.mult, ALU.add)
                s = sq.tile([P, BN], BF16, tag="s")
                nc.vector.tensor_mul(s[:], p[:], m[:])
                bq = sqf.tile([P, BN], BF16, tag="bq")
                nc.scalar.square(bq[:], m[:])
                a = sqf.tile([P, BN], BF16, tag="a")
                nc.scalar.square(a[:], p[:])
                t = sq.tile([P, BN], BF16, tag="t")
                nc.vector.tensor_sub(t[:], a[:], bq[:])
                return s, t

            # f2 chunks 0..2 derived up front (DVE is idle until ~14us);
            # the rest are derived between per-tile chains below.
            derive_f2(0)
            derive_f2(1)
            derive_f2(2)

            warm_fill(6)
            for b in range(NB):
                bs = slice(b * BN, (b + 1) * BN)
                zz = psz.tile([P, BN], F32, tag="zz")
                stq = []
                if b == 0:
                    # 2-tile software-pipelined prologue: P3/P1 of tiles
                    # 0,1 run before any P2 so the PE isn't head-of-line
                    # blocked on the laggy xti (SW-DGE) semaphore.
                    pr = []
                    for l in (0, 1):
                        p1 = pse.tile([P, BN], F32, tag="p1")
                        p2 = pse.tile([P, BN], F32, tag="p2")
                        p3 = pse.tile([P, BN], F32, tag="p3")
                        mm_stream(p3, f3, xts, 0, l)
                        c3 = tmp.tile([P, BN], F32, tag="c3")
                        nc.scalar.copy(c3[:], p3[:])
                        mm_stream(p1, f1, xtr, 0, l)
                        warm_fill(4)
                        pr.append((p1, p2, c3))
                    for l in (0, 1):
                        mm_stream(pr[l][1], f2, xti, 0, l)
                    for l in (0, 1):
                        s, t = dve_chain(*pr[l])
                        stq.extend((s, t))
                        if l == 1:
                            late_x([(xti, xti_d, 1)], s[:, 0:1])
                            derive_f2(3)
                            packs.append((zz, 0, 0, tuple(stq), bs))
                            stq = []
                lrange = range(2, NL) if b == 0 else range(NL)
                for l in lrange:
                    lastt = b == NB - 1 and l == NL - 1
                    p1 = pse.tile([P, BN], F32, tag="p1")
                    p2 = pse.tile([P, BN], F32, tag="p2")
                    p3 = pse.tile([P, BN], F32, tag="p3")
                    mm_stream(p3, f3, xts, b, l)
                    c3 = tmp.tile([P, BN], F32, tag="c3")
                    nc.scalar.copy(c3[:], p3[:])
                    mm_stream(p1, f1, xtr, b, l)
                    mm_stream(p2, f2, xti, b, l)

                    # On the last tile, first flush the (l6,l7) pack's
                    # l6 half (quadrants 0/1) -- its inputs are ready --
                    # so only the l7 half trails the final DVE chain.
                    if lastt:
                        s0, t0 = stq
                        for j, (wt, rhs, lw) in enumerate(
                                ((w1, s0, NL - 2), (w2, t0, NL - 2))):
                            nc.tensor.matmul(
                                zz[32 * j:32 * j + 32, :], wt[:, wsl(lw)],
                                rhs[:], start=False, stop=True,
                                tile_position=(0, 32 * j),
                                skip_group_check=True)
                        zt = zo.tile([P, BN], F32, tag="zt")
                        nc.scalar.copy(zt[0:64, :], zz[0:64, :])
                        nc.sync.dma_start(out_d[0:64, bs], zt[0:64, :])

                    margin = 0 if b == NB - 1 else 1
                    if len(packs) > margin:
                        zpack(*packs.pop(0))

                    s, t = dve_chain(p1, p2, c3)
                    if b == 0 and 2 <= l <= 5:
                        derive_f2(l + 2)
                    if b == 0 and l == 4:
                        late_x([(xtr, xtr_d, 2), (xti, xti_d, 2),
                                (xts, xts_d, 2)], s[:, 0:1])
                    elif b == 1 and l == 2:
                        late_x([(xtr, xtr_d, 3), (xti, xti_d, 3),
                                (xts, xts_d, 3)], s[:, 0:1])
                    if not lastt:
                        stq.extend((s, t))
                        if l % 2 == 1:
                            packs.append((zz, b, l // 2, tuple(stq), bs))
                            stq = []
                    else:
                        # l7 half: quadrants 2/3, then copy + DMA
                        for j, (wt, rhs) in enumerate(((w1, s), (w2, t))):
                            nc.tensor.matmul(
                                zz[64 + 32 * j:96 + 32 * j, :],
                                wt[:, wsl(NL - 1)], rhs[:],
                                start=False, stop=True,
                                tile_position=(0, 64 + 32 * j),
                                skip_group_check=True)
                        half = BN // 2
                        nc.scalar.copy(zt[64:128, 0:half], zz[64:128, 0:half])
                        nc.sync.dma_start(
                            out_d[64:128, bs.start:bs.start + half],
                            zt[64:128, 0:half])
                        nc.scalar.copy(zt[64:128, half:], zz[64:128, half:])
                        nc.sync.dma_start(
                            out_d[64:128, bs.start + half:bs.stop],
                            zt[64:128, half:])

            while packs:
                zpack(*packs.pop(0))

    nc.compile()
    _NC_CACHE = nc
    return nc


def _packW(a):
    """[1024, 20] -> [128, NL*32]: per l-chunk, rows l*128..(l+1)*128 land on
    partitions, the 20 cols zero-pad to 32; chunks stack along free dim."""
    padded = np.concatenate(
        [a, np.zeros((a.shape[0], WZ - a.shape[1]))], axis=1)
    return np.ascontiguousarray(
        padded.reshape(NL, P, WZ).transpose(1, 0, 2).reshape(P, -1))


def _packF(a):
    """[512, 1024] -> [128, 4096] l-major: col l*512 + d*128 + c holds
    F[d*128+p, l*128+c], so one l-chunk's 4 contraction slices are
    contiguous and can be DMA'd just ahead of their first use."""
    return np.ascontiguousarray(
        a.reshape(ND, P, NL, P).transpose(1, 2, 0, 3).reshape(P, -1))


def _host_weights(w0_real, w0_imag, wlast_real, wlast_imag):
    w0 = w0_real.astype(np.float64) + 1j * w0_imag.astype(np.float64)
    wl = wlast_real.astype(np.float64) + 1j * wlast_imag.astype(np.float64)
    F = np.fft.fft(w0, n=L, axis=1)
    Wt = np.fft.ifft(
        np.concatenate([wl, np.zeros((1, C))], axis=0), axis=0)
    bf = ml_dtypes.bfloat16
    F1 = _packF(F.real.astype(bf))
    F3 = _packF((F.real + F.imag).astype(bf))
    Wtr, Wti = Wt.real, Wt.imag
    W1 = _packW(np.hstack([Wtr, Wti])).astype(bf)
    W2 = _packW(np.hstack([-Wti, Wtr]) / 2.0).astype(bf)
    return F1, F3, W1, W2


def make_in_maps(x_real, x_imag, w0_real, w0_imag, wlast_real, wlast_imag):
    F1, F3, W1, W2 = _host_weights(
        w0_real, w0_imag, wlast_real, wlast_imag)
    bf = ml_dtypes.bfloat16
    xr = np.ascontiguousarray(x_real.T, dtype=bf)
    xi = np.ascontiguousarray(x_imag.T, dtype=bf)

    xsum = np.ascontiguousarray(
        (x_real.astype(np.float32) + x_imag.astype(np.float32)).T, dtype=bf)

    def pack3d(a):      # [512, BC] -> [128, NB, ND*BN], contiguous per b
        return np.ascontiguousarray(
            a.reshape(ND, P, NB, BN).transpose(1, 2, 0, 3).reshape(
                P, NB, ND * BN))

    in_maps = []
    for c in range(NCORES):
        sl = slice(c * BC, (c + 1) * BC)
        in_maps.append({
            "xT_r": pack3d(xr[:, sl]),
            "xT_i": pack3d(xi[:, sl]),
            "xT_s": pack3d(xsum[:, sl]),
            "F_1": F1, "F_3": F3,
            "W_1": W1, "W_2": W2,
        })
    return in_maps


def postprocess(results):
    outs = []
    for c in range(NCORES):
        o = results[c]["out"]
        # sum the 4 PE column-group quadrants, then |z|
        z = (o[0:2 * C] + o[32:32 + 2 * C]
             + o[64:64 + 2 * C] + o[96:96 + 2 * C])
        mag = np.sqrt(z[:C] ** 2 + z[C:2 * C] ** 2).T
        outs.append(mag)
    return np.ascontiguousarray(np.concatenate(outs, axis=0), dtype=np.float32)


def kernel(x_real, x_imag, w0_real, w0_imag, wlast_real, wlast_imag):
    x_real, x_imag, w0_real, w0_imag, wlast_real, wlast_imag = (
        np.asarray(arr) for arr in
        (x_real, x_imag, w0_real, w0_imag, wlast_real, wlast_imag))
    nc = build_nc()
    in_maps = make_in_maps(
        x_real, x_imag, w0_real, w0_imag, wlast_real, wlast_imag)
    # A stale/wedged NeuronCore (e.g. a previously killed process that died
    # mid-execute) fails with NRT_EXEC_UNIT_UNRECOVERABLE; reloading resets
    # it but may need a fresh backend and a moment. Retry a few times.
    import time
    last = None
    for attempt in range(4):
        try:
            res = run_bass_kernel_spmd(
                nc, in_maps, core_ids=list(range(NCORES)))
            return postprocess(res.results)
        except Exception as e:
            last = e
            time.sleep(2.0 + 2.0 * attempt)
            try:
                import jax
                import jax.extend.backend
                jax.clear_caches()
                jax.extend.backend.clear_backends()
            except Exception:
                pass
    raise last
